# revision 1
# baseline (speedup 1.0000x reference)
"""BoundaryAttentionModule Trainium2 kernel.

Shapes (hardcoded): b=4, c=256, h=w=64 (HW=4096), boundary 128x128,
mid=64, out_ch=256. 8 cores: core = (batch bi = core//2, key-half kh = core%2).

Math (exact reassociation of the reference):
  bm   = nearest-downsampled boundary map        [b, 4096]
  R    = relu(kw1f outer bm_khalf + beta)        [64, 2048]   (kw1f = key_w1*bn_inv)
  G    = (key_w2^T @ query_w) @ u                [64, 4096]
  E^T  = R^T @ G                                 [2048_k, 4096_j]  (logits tiny, no max sub)
  U    = exp(E^T), s[k] = sum_j U[k, j]
  Vt   = (u^T @ value_w^T)[k_half] / s * 8192    [2048, 256]
  P    = Vt^T @ U                                [256, 4096]  per-core partial (x8192)
host: out[bi] = (gamma/8192) * (P[2bi] + P[2bi+1]) + u[bi]

Energy matmuls in bf16 with K=64 contraction packed as concurrent
partition-half duos (R and G are host/device-duplicated into both
partition halves, so two j-slices of one k-tile run in the PE array
simultaneously).  The output matmul runs in fp8e4 DoubleRow (2 keys per
cell); the key axis is host-permuted pairwise so PSUM partitions
interleave — the permutation only reorders the contracted axis.
"""

import numpy as np

B, C, HW = 4, 256, 4096
KH = HW // 2          # 2048 keys per core
NK = KH // 128        # 16 k tiles
NP = NK // 2          # 8 k-tile pairs
MID = 64
VSCALE = 8192.0       # fp8 scaling of Vt (power of two; host divides gamma)

TRACE = False
TRACE_CORES = None
LAST_RESULTS = None

_BUILT = None


def _build():
    import concourse.bass as bass
    import concourse.tile as tile
    from concourse import bacc, mybir

    f32 = mybir.dt.float32
    bf16 = mybir.dt.bfloat16
    fp8 = mybir.dt.float8e4
    AF = mybir.ActivationFunctionType
    AX = mybir.AxisListType
    ALU = mybir.AluOpType

    nc = bacc.Bacc(
        "TRN2",
        target_bir_lowering=False,
        debug=False,
        enable_asserts=False,
        num_devices=8,
    )

    u_in = nc.dram_tensor("u_in", [C, HW], bf16, kind="ExternalInput").ap()
    uk_in = nc.dram_tensor("uk_in", [C, KH], bf16, kind="ExternalInput").ap()
    bmk_in = nc.dram_tensor("bmk_in", [1, KH], bf16, kind="ExternalInput").ap()
    # M2^T = [M; M]^T  [256, 128]  (duplicated so G lands in both halves)
    mt_in = nc.dram_tensor("mt_in", [C, 2 * MID], bf16, kind="ExternalInput").ap()
    vwt_in = nc.dram_tensor("vwt_in", [C, C], bf16, kind="ExternalInput").ap()
    kw1f_in = nc.dram_tensor("kw1f_in", [1, 2 * MID], bf16, kind="ExternalInput").ap()
    beta_in = nc.dram_tensor("beta_in", [2 * MID, 1], f32, kind="ExternalInput").ap()
    out_d = nc.dram_tensor("outp", [C, HW], f32, kind="ExternalOutput").ap()

    # j-chunking of the 4096 axis: two 1536 chunks + one 1024 chunk.
    # PSUM: "big" slots [128,1536] (3 banks) x2 bufs + "small" (1 bank) x2 = 8.
    CHUNKS = [(0, 1536), (1536, 1536), (3072, 1024)]
    C_CHUNKS = CHUNKS

    with tile.TileContext(nc) as tc:
        with (
            tc.tile_pool(name="sb", bufs=1) as sb,
            tc.tile_pool(name="ost", bufs=2) as osp,
            tc.tile_pool(name="ps", bufs=2, space="PSUM") as ps,
        ):
            # ---- weights / inputs; u on sync queue, the rest on gpsimd ----
            mt = sb.tile([128, 2 * MID], bf16, tag="mt", name="mt")
            nc.gpsimd.dma_start(mt[0:128, :], mt_in[0:128, :])
            mt1 = sb.tile([128, 2 * MID], bf16, tag="mt1", name="mt1")
            nc.gpsimd.dma_start(mt1[0:128, :], mt_in[128:256, :])
            kw1 = sb.tile([1, 2 * MID], bf16, tag="kw1", name="kw1")
            nc.gpsimd.dma_start(kw1[:], kw1f_in[:, :])
            betat = sb.tile([2 * MID, 1], f32, tag="betat", name="betat")
            nc.gpsimd.dma_start(betat[:], beta_in[:, :])
            bmk = sb.tile([1, KH], bf16, tag="bmk", name="bmk")
            nc.gpsimd.dma_start(bmk[:], bmk_in[:, :])
            u0 = sb.tile([128, HW], bf16, tag="u0", name="u0")
            u1 = sb.tile([128, HW], bf16, tag="u1", name="u1")
            for jo, w in CHUNKS:
                nc.sync.dma_start(u0[:, jo : jo + w], u_in[0:128, jo : jo + w])
                nc.sync.dma_start(u1[:, jo : jo + w], u_in[128:256, jo : jo + w])
            vwt0 = sb.tile([128, C], bf16, tag="vwt0", name="vwt0")
            nc.gpsimd.dma_start(vwt0[:], vwt_in[0:128, :])
            vwt1 = sb.tile([128, C], bf16, tag="vwt1", name="vwt1")
            nc.gpsimd.dma_start(vwt1[:], vwt_in[128:256, :])
            uk0 = sb.tile([128, KH], bf16, tag="uk0", name="uk0")
            nc.gpsimd.dma_start(uk0[:], uk_in[0:128, :])
            uk1 = sb.tile([128, KH], bf16, tag="uk1", name="uk1")
            nc.gpsimd.dma_start(uk1[:], uk_in[128:256, :])

            # ---- R2 = relu(kw1f2 outer bmk + beta2): both halves [128, 2048] ----
            R2 = sb.tile([128, KH], bf16, tag="R2", name="R2")
            for rc in range(KH // 512):
                pr = ps.tile([128, 512], f32, tag="small", bufs=2, name=f"pr{rc}")
                nc.tensor.matmul(
                    pr[:], kw1[:, :], bmk[:, rc * 512 : (rc + 1) * 512],
                    start=True, stop=True,
                )
                nc.scalar.activation(
                    R2[:, rc * 512 : (rc + 1) * 512], pr[:], AF.Relu,
                    bias=betat[:, 0:1],
                )

            # ---- G2 = M2 @ u (both halves) + first k-tile energy interleaved ----
            G2 = sb.tile([128, HW], bf16, tag="G2", name="G2")
            s_all = sb.tile([128, NK], f32, tag="s_all", name="s_all")
            rinv_all = sb.tile([128, NK], f32, tag="rinv", name="rinv_all")
            sp_tail = {}
            for kt in range(NK - 3, NK):
                sp_tail[kt] = sb.tile([128, 4], f32, tag=f"sp{kt}", name=f"sp{kt}")
            u_pairs = []
            for pair in range(NP):
                Up = sb.tile([128, 2 * HW], fp8, tag=f"Up{pair}", name=f"Up{pair}")
                u_pairs.append(Up)
            vtb = []
            for kt in range(NK):
                v = sb.tile([128, C], bf16, tag=f"vtb{kt}", name=f"vtb{kt}")
                vtb.append(v)
            vtsp = []
            for pair in range(NP):
                vp = sb.tile([128, 2 * C], fp8, tag=f"vtsp{pair}", name=f"vtsp{pair}")
                vtsp.append(vp)

            N_ACC = 3  # last k-tiles whose row-sum rides the ACT accumulator

            def energy_chunk(kt, ci):
                """Energy matmuls + exp for one (k-tile, j-chunk)."""
                pair, half = kt // 2, kt % 2
                Up = u_pairs[pair]
                accum_tail = kt >= NK - N_ACC
                jo, w = CHUNKS[ci]
                pe = ps.tile([128, 1536], f32, tag="big", name=f"pe{kt}_{jo}")
                nq = w // 512
                for q in range(0, nq, 2):
                    # concurrent partition-half duo (K=64 row groups)
                    js0 = jo + q * 512
                    nc.tensor.matmul(
                        pe[:, q * 512 : (q + 1) * 512],
                        R2[0:64, kt * 128 : (kt + 1) * 128],
                        G2[0:64, js0 : js0 + 512],
                        start=True, stop=True,
                    )
                    if q + 1 < nq:
                        js1 = jo + (q + 1) * 512
                        nc.tensor.matmul(
                            pe[:, (q + 1) * 512 : (q + 2) * 512],
                            R2[64:128, kt * 128 : (kt + 1) * 128],
                            G2[64:128, js1 : js1 + 512],
                            start=True, stop=True,
                        )
                nc.scalar.activation(
                    Up[:, half * HW + jo : half * HW + jo + w],
                    pe[:, 0:w], AF.Exp,
                    accum_out=(sp_tail[kt][:, ci : ci + 1] if accum_tail else None),
                )
                if accum_tail and ci == len(CHUNKS) - 1:
                    nc.vector.reduce_sum(
                        s_all[:, kt : kt + 1], sp_tail[kt][:, 0:3], axis=AX.X
                    )

            def ktile_epilogue(kt):
                """Row-sum (if not ACT-accumulated) + Vt matmul pair + scales."""
                pair, half = kt // 2, kt % 2
                if kt < NK - N_ACC:
                    nc.vector.reduce_sum(
                        s_all[:, kt : kt + 1],
                        u_pairs[pair][:, half * HW : (half + 1) * HW], axis=AX.X,
                    )
                pv = ps.tile([128, C], f32, tag="small", bufs=2, name=f"pv{kt}")
                ko = kt * 128
                nc.tensor.matmul(
                    pv[:], uk0[:, ko : ko + 128], vwt0[:, :], start=True, stop=False
                )
                nc.tensor.matmul(
                    pv[:], uk1[:, ko : ko + 128], vwt1[:, :], start=False, stop=True
                )
                nc.vector.tensor_copy(vtb[kt][:], pv[:])
                if half == 1:
                    nc.vector.reciprocal(
                        rinv_all[:, kt - 1 : kt + 1], s_all[:, kt - 1 : kt + 1]
                    )
                    for h2 in (0, 1):
                        nc.gpsimd.tensor_scalar(
                            vtsp[pair][:, h2 * C : (h2 + 1) * C],
                            vtb[kt - 1 + h2][:],
                            rinv_all[:, kt - 1 + h2 : kt + h2], VSCALE,
                            op0=ALU.mult, op1=ALU.mult,
                        )

            # G chunk production interleaved chunk-major with k-tiles 0 and 1,
            # so ACT has exp work while later G chunks are still being built
            for ci, (jo, w) in enumerate(CHUNKS):
                pg = ps.tile([128, 1536], f32, tag="big", name=f"pg{jo}")
                for q in range(w // 512):
                    sl = slice(q * 512, (q + 1) * 512)
                    js = jo + q * 512
                    nc.tensor.matmul(
                        pg[:, sl], mt[:, :], u0[:, js : js + 512],
                        start=True, stop=False,
                    )
                    nc.tensor.matmul(
                        pg[:, sl], mt1[:, :], u1[:, js : js + 512],
                        start=False, stop=True,
                    )
                nc.vector.tensor_copy(G2[:, jo : jo + w], pg[:, 0:w])
                energy_chunk(0, ci)
                energy_chunk(1, ci)
            ktile_epilogue(0)
            ktile_epilogue(1)
            for kt in range(2, NK):
                for ci in range(len(CHUNKS)):
                    energy_chunk(kt, ci)
                ktile_epilogue(kt)

            # ---- P = Vt^T @ U  (fp8 DoubleRow: 2 keys/cell) -> DRAM ----
            DR = mybir.MatmulPerfMode.DoubleRow
            for ct in range(2):
                for jg, (jo, w) in enumerate(C_CHUNKS):
                    po = ps.tile([128, 1536], f32, tag="big", name=f"po{ct}_{jg}")
                    for pair in range(NP):
                        lhsT = vtsp[pair].rearrange("p (i c) -> p i c", i=2)[
                            :, :, ct * 128 : (ct + 1) * 128
                        ]
                        for q in range(w // 512):
                            sl = slice(q * 512, (q + 1) * 512)
                            js = jo + q * 512
                            rhs = u_pairs[pair].rearrange("p (i j) -> p i j", i=2)[
                                :, :, js : js + 512
                            ]
                            nc.tensor.matmul(
                                po[:, sl], lhsT, rhs,
                                start=(pair == 0), stop=(pair == NP - 1),
                                perf_mode=DR,
                            )
                    ost = osp.tile([128, 1536], f32, tag="ost", name=f"ost{ct}_{jg}")
                    if ct == 1 and jg == len(C_CHUNKS) - 1:
                        # final group: split copy/DMA halves to shorten the tail
                        h = w // 2
                        nc.scalar.copy(ost[:, 0:h], po[:, 0:h])
                        nc.sync.dma_start(
                            out_d[ct * 128 : (ct + 1) * 128, jo : jo + h],
                            ost[:, 0:h],
                        )
                        nc.scalar.copy(ost[:, h:w], po[:, h:w])
                        nc.scalar.dma_start(
                            out_d[ct * 128 : (ct + 1) * 128, jo + h : jo + w],
                            ost[:, h:w],
                        )
                    else:
                        nc.scalar.copy(ost[:, 0:w], po[:, 0:w])
                        nc.sync.dma_start(
                            out_d[ct * 128 : (ct + 1) * 128, jo : jo + w],
                            ost[:, 0:w],
                        )

    nc.compile()
    return nc


def _get_built():
    global _BUILT
    if _BUILT is None:
        _BUILT = _build()
    return _BUILT


def _kperm():
    """Pairwise interleave within 256-key blocks: new index kt*128+q maps to
    old key  (kt//2)*256 + 2q + (kt%2)."""
    perm = np.empty(KH, np.int64)
    for pair in range(NP):
        base = pair * 256
        perm[pair * 256 : pair * 256 + 128] = base + np.arange(0, 256, 2)
        perm[pair * 256 + 128 : pair * 256 + 256] = base + np.arange(1, 256, 2)
    return perm


def _host_prep(boundary_map, uncertainty_map, key_w1, bn_scale, bn_bias,
               bn_mean, bn_var, key_w2, query_w, value_w):
    import ml_dtypes

    bf16 = ml_dtypes.bfloat16
    b, c, h, w = uncertainty_map.shape
    H0 = boundary_map.shape[2]
    idx = (np.arange(h) * H0) // h
    bm = boundary_map[:, 0][:, idx][:, :, idx].reshape(b, h * w).astype(np.float32)

    inv = bn_scale / np.sqrt(bn_var + 1e-5)
    beta = (bn_bias - bn_mean * inv).astype(np.float32)
    kw1f = (key_w1[:, 0] * inv).astype(np.float32)
    m_t = np.ascontiguousarray((key_w2.T @ query_w).T).astype(np.float32)  # [256, 64]
    # duplicate across partition halves for the energy duo-packing
    kw1f2 = np.concatenate([kw1f, kw1f]).reshape(1, 2 * MID).astype(bf16)
    beta2 = np.concatenate([beta, beta]).reshape(2 * MID, 1).astype(np.float32)
    m_t2 = np.concatenate([m_t, m_t], axis=1).astype(bf16)                 # [256, 128]
    vw_t = np.ascontiguousarray(value_w.T).astype(bf16)                    # [256, 256]
    perm = _kperm()

    in_maps = []
    for core in range(8):
        bi, kh = core // 2, core % 2
        u = np.ascontiguousarray(uncertainty_map[bi].reshape(c, h * w)).astype(bf16)
        uk = u[:, kh * KH : (kh + 1) * KH][:, perm]
        bmk = bm[bi, kh * KH : (kh + 1) * KH][perm]
        in_maps.append({
            "u_in": u,
            "uk_in": np.ascontiguousarray(uk),
            "bmk_in": np.ascontiguousarray(bmk).reshape(1, KH).astype(bf16),
            "mt_in": m_t2,
            "vwt_in": vw_t,
            "kw1f_in": kw1f2,
            "beta_in": beta2,
        })
    return in_maps


def kernel(boundary_map, uncertainty_map, key_w1, bn_scale, bn_bias,
           bn_mean, bn_var, key_w2, query_w, value_w, gamma):
    global LAST_RESULTS
    from concourse.bass_utils import run_bass_kernel_spmd

    nc = _get_built()
    in_maps = _host_prep(
        np.asarray(boundary_map), np.asarray(uncertainty_map), np.asarray(key_w1),
        np.asarray(bn_scale), np.asarray(bn_bias), np.asarray(bn_mean),
        np.asarray(bn_var), np.asarray(key_w2), np.asarray(query_w),
        np.asarray(value_w),
    )
    kwargs = {}
    if TRACE:
        kwargs["trace"] = True
        if TRACE_CORES is not None:
            kwargs["trace_cores"] = TRACE_CORES
    res = run_bass_kernel_spmd(nc, in_maps, core_ids=list(range(8)), **kwargs)
    LAST_RESULTS = res

    b, c, h, w = uncertainty_map.shape
    g = np.float32(np.asarray(gamma).reshape(-1)[0] / VSCALE)
    out = np.empty((b, c, h * w), np.float32)
    um = np.asarray(uncertainty_map)
    for bi in range(b):
        P = res.results[2 * bi]["outp"] + res.results[2 * bi + 1]["outp"]
        out[bi] = g * P + um[bi].reshape(c, h * w)
    return out.reshape(b, c, h, w)



# revision 4
# speedup vs baseline: 2.7649x; 2.7649x over previous
"""BoundaryAttentionModule Trainium2 kernel — segment-decomposition rewrite.

Shapes (hardcoded): b=4, c=256, h=w=64 (HW=4096), boundary 128x128,
mid=64, out_ch=256. 8 cores: core = (batch bi = core//2, key-half kh = core%2).

Key observation: the energy E[k,j] = relu(kw1f*t_k + beta)^T G[:,j] depends on
key k only through the scalar t_k = bm[k], and relu makes E piecewise-LINEAR
in t with <=64 breakpoints (26 fall inside the data range).  Split the t-range
into S=64 pseudo-segments (within true relu segments).  For each pseudo-segment
evaluate the energy row at its two edge anchors (128 rows total):

  E2[2s+a, j] = (t_anchor * MA[seg] + MB[seg]) @ G[:, j] = (ME2 @ M) @ u
  Fcat        = exp(E2)                              [128, 4096]

Every key's attention row is then a host-known convex combination of its
segment's two anchor rows (linear interp in t; exact up to (width*|A|)^2/8,
~1e-5 relative here):

  exp(E[k, :]) ~= wl_k * Fcat[2s_k, :] + wh_k * Fcat[2s_k+1, :]
  s_k  = wl_k * SA[2s_k] + wh_k * SA[2s_k+1]        (SA = Fcat row sums)
  P    = W^T @ Fcat,   W[r, c] = sum_k wmask[k, r]/s_k * Vt[k, c]

This removes the [2048, 4096] energy matmul, shrinks exp from 8.4M to 0.5M
elements, and cuts the output-matmul contraction from 2048 keys to 128 rows.
All device arithmetic is bf16 (f32 PSUM); no fp8 needed.

host: out[bi] = gamma * (P[2bi] + P[2bi+1]) + u[bi]
"""

import numpy as np

B, C, HW = 4, 256, 4096
KH = HW // 2          # 2048 keys per core
NK = KH // 128        # 16 k-tiles
S = 64                # pseudo-segments
NR = 2 * S            # 128 anchor rows

TRACE = False
TRACE_CORES = None
LAST_RESULTS = None

_BUILT = None


def _build():
    import concourse.bass as bass
    import concourse.tile as tile
    from concourse import bacc, mybir

    f32 = mybir.dt.float32
    bf16 = mybir.dt.bfloat16
    AF = mybir.ActivationFunctionType
    AX = mybir.AxisListType
    ALU = mybir.AluOpType

    nc = bacc.Bacc(
        "TRN2",
        target_bir_lowering=False,
        debug=False,
        enable_asserts=False,
        num_devices=8,
    )

    u_in = nc.dram_tensor("u_in", [C, HW], bf16, kind="ExternalInput").ap()
    me2mt_in = nc.dram_tensor("me2mt_in", [C, NR], bf16, kind="ExternalInput").ap()
    vwt_in = nc.dram_tensor("vwt_in", [C, C], bf16, kind="ExternalInput").ap()
    # wmask_in[p, kt*128 + r] = wmask[kt*128 + p, r]   (k-tile-major)
    wmask_in = nc.dram_tensor("wmask_in", [128, NK * NR], bf16, kind="ExternalInput").ap()
    # gw_in[r, kt*128 + kk] = wmask[kt*128 + kk, r]    (transposed blocks)
    gw_in = nc.dram_tensor("gw_in", [NR, KH], bf16, kind="ExternalInput").ap()
    out_d = nc.dram_tensor("outp", [C, HW], bf16, kind="ExternalOutput").ap()

    with tile.TileContext(nc) as tc:
        with (
            tc.tile_pool(name="sb", bufs=1) as sb,
            tc.tile_pool(name="ps", bufs=1, space="PSUM") as ps,
        ):
            # ---- tiny tiles + dummy exp to prepay the ACT table load ----
            dsrc = sb.tile([128, 1], bf16, tag="dsrc", name="dsrc")
            ddst = sb.tile([128, 1], bf16, tag="ddst", name="ddst")
            nc.vector.memset(dsrc[:], 0.0)
            nc.scalar.activation(ddst[:], dsrc[:], AF.Exp)

            # ---- input DMAs: u0 on sync, u1 + me2 on vector, rest gpsimd ----
            me2a = sb.tile([128, NR], bf16, tag="me2a", name="me2a")
            me2b = sb.tile([128, NR], bf16, tag="me2b", name="me2b")
            nc.scalar.dma_start(me2a[:], me2mt_in[0:128, :])
            nc.scalar.dma_start(me2b[:], me2mt_in[128:256, :])
            vwt0 = sb.tile([128, C], bf16, tag="vwt0", name="vwt0")
            nc.gpsimd.dma_start(vwt0[:], vwt_in[0:128, :])
            vwt1 = sb.tile([128, C], bf16, tag="vwt1", name="vwt1")
            nc.gpsimd.dma_start(vwt1[:], vwt_in[128:256, :])
            u0 = sb.tile([128, HW], bf16, tag="u0", name="u0")
            u1 = sb.tile([128, HW], bf16, tag="u1", name="u1")
            for ci in range(4):
                sl = slice(ci * 1024, (ci + 1) * 1024)
                nc.sync.dma_start(u0[:, sl], u_in[0:128, sl])
                nc.scalar.dma_start(u1[:, sl], u_in[128:256, sl])
            wmask = sb.tile([128, NK * NR], bf16, tag="wmask", name="wmask")
            nc.gpsimd.dma_start(wmask[:], wmask_in[:, :])
            gw = sb.tile([NR, KH], bf16, tag="gw", name="gw")
            nc.gpsimd.dma_start(gw[:], gw_in[:, :])

            Fcat = sb.tile([128, HW], bf16, tag="Fcat", name="Fcat")
            SAp = sb.tile([128, 4], f32, tag="SAp", name="SAp")
            SAv = sb.tile([128, 1], f32, tag="SAv", name="SAv")
            SAb = sb.tile([128, 1], bf16, tag="SAb", name="SAb")
            rinv = sb.tile([128, NK], f32, tag="rinv", name="rinv")
            vtb = sb.tile([128, NK * C], bf16, tag="vtb", name="vtb")
            wsc = sb.tile([128, NK * NR], bf16, tag="wsc", name="wsc")
            Wsb = sb.tile([128, C], bf16, tag="Wsb", name="Wsb")
            osb0 = sb.tile([128, HW], bf16, tag="osb0", name="osb0")
            osb1 = sb.tile([128, HW], bf16, tag="osb1", name="osb1")

            # ---- phase 1: E2 + exp (all j) and Vt (key half = cols 0:2048) ----
            for ci in range(4):
                e2 = ps.tile([128, 1024], f32, tag="big", bufs=2, name=f"e2_{ci}")
                for q in range(2):
                    sl = slice(ci * 1024 + q * 512, ci * 1024 + (q + 1) * 512)
                    nc.tensor.matmul(
                        e2[:, q * 512 : (q + 1) * 512], me2a[:], u0[:, sl],
                        start=True, stop=False,
                    )
                    nc.tensor.matmul(
                        e2[:, q * 512 : (q + 1) * 512], me2b[:], u1[:, sl],
                        start=False, stop=True,
                    )
                nc.scalar.activation(
                    Fcat[:, ci * 1024 : (ci + 1) * 1024], e2[:, 0:1024], AF.Exp,
                    accum_out=SAp[:, ci : ci + 1],
                )
                if ci < 2:
                    # 8 k-tiles live in this chunk; 2 per PSUM buffer
                    for g in range(4):
                        vt = ps.tile([128, 512], f32, tag="vt", bufs=2,
                                     name=f"vt_{ci}_{g}")
                        for t2 in range(2):
                            kt = ci * 8 + g * 2 + t2
                            ko = kt * 128
                            nc.tensor.matmul(
                                vt[:, t2 * 256 : (t2 + 1) * 256],
                                u0[:, ko : ko + 128], vwt0[:],
                                start=True, stop=False,
                            )
                            nc.tensor.matmul(
                                vt[:, t2 * 256 : (t2 + 1) * 256],
                                u1[:, ko : ko + 128], vwt1[:],
                                start=False, stop=True,
                            )
                        kt0 = ci * 8 + g * 2
                        nc.vector.tensor_copy(
                            vtb[:, kt0 * 256 : (kt0 + 2) * 256], vt[:]
                        )

            # ---- softmax denominators: SA -> gather -> 1/s ----
            nc.vector.reduce_sum(SAv[:], SAp[:, 0:4], axis=AX.X)
            nc.vector.tensor_copy(SAb[:], SAv[:])
            sg = ps.tile([128, NK], f32, tag="sg", name="sg")
            for kt in range(NK):
                nc.tensor.matmul(
                    sg[:, kt : kt + 1], gw[:, kt * 128 : (kt + 1) * 128], SAb[:],
                    start=True, stop=True,
                )
            nc.vector.reciprocal(rinv[:], sg[:])

            # ---- W = (wmask/s)^T @ Vt   [128 rows, 256 c] ----
            wps = ps.tile([128, C], f32, tag="w", name="wps")
            for kt in range(NK):
                nc.vector.tensor_scalar(
                    wsc[:, kt * NR : (kt + 1) * NR],
                    wmask[:, kt * NR : (kt + 1) * NR],
                    rinv[:, kt : kt + 1], None, op0=ALU.mult,
                )
                nc.tensor.matmul(
                    wps[:], wsc[:, kt * NR : (kt + 1) * NR],
                    vtb[:, kt * C : (kt + 1) * C],
                    start=(kt == 0), stop=(kt == NK - 1),
                )
            nc.scalar.copy(Wsb[:], wps[:])

            # ---- P = W^T @ Fcat -> bf16 -> DRAM ----
            for ct in range(2):
                osb = osb0 if ct == 0 else osb1
                for ci in range(4):
                    pp = ps.tile([128, 1024], f32, tag="big", bufs=2,
                                 name=f"pp_{ct}_{ci}")
                    for q in range(2):
                        sl = slice(ci * 1024 + q * 512, ci * 1024 + (q + 1) * 512)
                        nc.tensor.matmul(
                            pp[:, q * 512 : (q + 1) * 512],
                            Wsb[:, ct * 128 : (ct + 1) * 128], Fcat[:, sl],
                            start=True, stop=True,
                        )
                    sl = slice(ci * 1024, (ci + 1) * 1024)
                    if (ct * 4 + ci) % 2 == 0:
                        nc.scalar.copy(osb[:, sl], pp[:])
                    else:
                        nc.vector.tensor_copy(osb[:, sl], pp[:])
                    qeng = nc.sync if ct == 0 else nc.gpsimd
                    qeng.dma_start(
                        out_d[ct * 128 : (ct + 1) * 128, sl], osb[:, sl]
                    )

    nc.compile()
    return nc


def _get_built():
    global _BUILT
    if _BUILT is None:
        _BUILT = _build()
    return _BUILT


def _prep_global(boundary_map, key_w1, bn_scale, bn_bias, bn_mean, bn_var,
                 key_w2, query_w, value_w):
    """Segment geometry + weight folding (all float64 host math)."""
    b = boundary_map.shape[0]
    h = 64
    H0 = boundary_map.shape[2]
    idx = (np.arange(h) * H0) // h
    bm = boundary_map[:, 0][:, idx][:, :, idx].reshape(b, HW).astype(np.float64)

    inv = bn_scale.astype(np.float64) / np.sqrt(bn_var.astype(np.float64) + 1e-5)
    beta = bn_bias.astype(np.float64) - bn_mean.astype(np.float64) * inv
    kw1f = key_w1[:, 0].astype(np.float64) * inv
    M = key_w2.astype(np.float64).T @ query_w.astype(np.float64)   # [64, 256]

    tall = bm.reshape(-1)
    lo, hi = tall.min(), tall.max()
    with np.errstate(divide="ignore", invalid="ignore"):
        tstar = np.where(np.abs(kw1f) > 1e-30, -beta / kw1f, np.inf)
    bps = np.sort(tstar[(tstar > lo) & (tstar < hi)])
    edges_true = np.concatenate([[lo], bps, [hi]])
    nseg = len(edges_true) - 1
    assert nseg <= S, f"too many relu segments: {nseg}"

    widths = np.diff(edges_true)
    cnt = np.histogram(tall, bins=edges_true)[0]
    score = widths * np.sqrt(cnt + 1)
    alloc = np.maximum(1, np.floor(score / score.sum() * S).astype(int))
    while alloc.sum() < S:
        alloc[np.argmax(score / alloc)] += 1
    while alloc.sum() > S:
        cand = np.where(alloc > 1)[0]
        alloc[cand[np.argmin((score / alloc)[cand])]] -= 1

    ps_lo = np.empty(S); ps_hi = np.empty(S); ps_true = np.empty(S, np.int64)
    si_out = 0
    for si in range(nseg):
        sub = np.linspace(edges_true[si], edges_true[si + 1], alloc[si] + 1)
        for k in range(alloc[si]):
            ps_lo[si_out] = sub[k]; ps_hi[si_out] = sub[k + 1]
            ps_true[si_out] = si
            si_out += 1
    assert si_out == S

    mids = 0.5 * (edges_true[:-1] + edges_true[1:])
    act = (kw1f[None, :] * mids[:, None] + beta[None, :]) > 0     # [nseg, 64]
    ME2 = np.zeros((NR, 64))
    for s in range(S):
        si = ps_true[s]
        for a, ta in enumerate((ps_lo[s], ps_hi[s])):
            ME2[2 * s + a] = ta * (act[si] * kw1f) + act[si] * beta
    me2m = ME2 @ M                                                # [128, 256]

    # per-key segment + interp weights, per batch
    seg = np.clip(np.searchsorted(ps_hi, bm, side="left"), 0, S - 1)  # [b, HW]
    width = np.maximum(ps_hi[seg] - ps_lo[seg], 1e-12)
    wl = (ps_hi[seg] - bm) / width
    wh = 1.0 - wl
    return bm, me2m, seg, wl, wh


def _host_prep(boundary_map, uncertainty_map, key_w1, bn_scale, bn_bias,
               bn_mean, bn_var, key_w2, query_w, value_w):
    import ml_dtypes

    bf16 = ml_dtypes.bfloat16
    b, c, h, w = uncertainty_map.shape
    _, me2m, seg, wl, wh = _prep_global(
        boundary_map, key_w1, bn_scale, bn_bias, bn_mean, bn_var,
        key_w2, query_w, value_w,
    )
    me2mt = np.ascontiguousarray(me2m.T).astype(bf16)             # [256, 128]
    vw_t = np.ascontiguousarray(value_w.T).astype(bf16)           # [256, 256]

    in_maps = []
    for core in range(8):
        bi, kh = core // 2, core % 2
        u = uncertainty_map[bi].reshape(c, h * w).astype(np.float32)
        if kh == 1:
            u = np.concatenate([u[:, KH:], u[:, :KH]], axis=1)
        ksl = slice(kh * KH, (kh + 1) * KH)
        sg_k = seg[bi, ksl]
        wmask = np.zeros((KH, NR), np.float32)
        kk = np.arange(KH)
        wmask[kk, 2 * sg_k] = wl[bi, ksl]
        wmask[kk, 2 * sg_k + 1] = wh[bi, ksl]
        # k-tile-major layouts
        wm_dev = np.ascontiguousarray(
            wmask.reshape(NK, 128, NR).transpose(1, 0, 2).reshape(128, NK * NR)
        ).astype(bf16)
        gw_dev = np.ascontiguousarray(
            wmask.reshape(NK, 128, NR).transpose(2, 0, 1).reshape(NR, KH)
        ).astype(bf16)
        in_maps.append({
            "u_in": np.ascontiguousarray(u).astype(bf16),
            "me2mt_in": me2mt,
            "vwt_in": vw_t,
            "wmask_in": wm_dev,
            "gw_in": gw_dev,
        })
    return in_maps


def kernel(boundary_map, uncertainty_map, key_w1, bn_scale, bn_bias,
           bn_mean, bn_var, key_w2, query_w, value_w, gamma):
    global LAST_RESULTS
    from concourse.bass_utils import run_bass_kernel_spmd

    nc = _get_built()
    in_maps = _host_prep(
        np.asarray(boundary_map), np.asarray(uncertainty_map), np.asarray(key_w1),
        np.asarray(bn_scale), np.asarray(bn_bias), np.asarray(bn_mean),
        np.asarray(bn_var), np.asarray(key_w2), np.asarray(query_w),
        np.asarray(value_w),
    )
    kwargs = {}
    if TRACE:
        kwargs["trace"] = True
        if TRACE_CORES is not None:
            kwargs["trace_cores"] = TRACE_CORES
    res = run_bass_kernel_spmd(nc, in_maps, core_ids=list(range(8)), **kwargs)
    LAST_RESULTS = res

    b, c, h, w = uncertainty_map.shape
    g = np.float32(np.asarray(gamma).reshape(-1)[0])
    out = np.empty((b, c, h * w), np.float32)
    um = np.asarray(uncertainty_map)
    for bi in range(b):
        P0 = res.results[2 * bi]["outp"].astype(np.float32)
        P1 = res.results[2 * bi + 1]["outp"].astype(np.float32)
        P1 = np.concatenate([P1[:, KH:], P1[:, :KH]], axis=1)
        out[bi] = g * (P0 + P1) + um[bi].reshape(c, h * w)
    return out.reshape(b, c, h, w)


# revision 5
# speedup vs baseline: 3.2593x; 1.1788x over previous
"""BoundaryAttentionModule Trainium2 kernel — segment decomposition, fp8 DR.

Shapes (hardcoded): b=4, c=256, h=w=64 (HW=4096), boundary 128x128,
mid=64, out_ch=256. 8 cores: core = (batch bi = core//2, key-half kh = core%2).

The energy E[k,j] = relu(kw1f*t_k + beta)^T G[:,j] depends on key k only
through the scalar t_k = bm[k], piecewise-linear in t with <=64 relu
breakpoints.  S=64 pseudo-segments x 2 edge anchors -> 128 rows:

  E2   = (ME2 @ M) @ u          [128, 4096]   (one matmul from u)
  Fcat = exp(E2 / SC)           [128, 4096]   (ME2M shipped x SC for fp8)
  exp(E[k,:]) ~= wl_k*Fcat[2s_k,:] + wh_k*Fcat[2s_k+1,:]   (host-known wl/wh)
  s_k  = wl_k*SA[2s_k] + wh_k*SA[2s_k+1],  SA = Fcat row sums
  P    = W^T @ Fcat,  W[r,c] = sum_k wmask[k,r]/s_k * Vt[k,c]

u / ME2M / value_w^T travel in fp8e4 (ME2M, vwt pre-scaled x64 to clear the
e4m3 denormal floor; host divides gamma by 64).  Channel axis is pairwise
interleaved so E2 and Vt run in DoubleRow mode (256-deep contraction per
pass).  Masks/everything else bf16, PSUM f32.

host: out[bi] = gamma/SC * (P[2bi] + P[2bi+1]) + u[bi]
"""

import numpy as np

B, C, HW = 4, 256, 4096
KH = HW // 2          # 2048 keys per core
NK = KH // 128        # 16 k-tiles
S = 64                # pseudo-segments
NR = 2 * S            # 128 anchor rows
SC = 64.0             # fp8 pre-scale on ME2M / vwt

TRACE = False
TRACE_CORES = None
LAST_RESULTS = None

_BUILT = None


def _build():
    import concourse.bass as bass
    import concourse.tile as tile
    from concourse import bacc, mybir

    f32 = mybir.dt.float32
    bf16 = mybir.dt.bfloat16
    fp8 = mybir.dt.float8e4
    AF = mybir.ActivationFunctionType
    AX = mybir.AxisListType
    ALU = mybir.AluOpType
    DR = mybir.MatmulPerfMode.DoubleRow

    nc = bacc.Bacc(
        "TRN2",
        target_bir_lowering=False,
        debug=False,
        enable_asserts=False,
        num_devices=8,
    )

    # u chunks: row block ci*128+p, col i*1024+x  <->  u8[2p+i, ci*1024+x]
    u_in = nc.dram_tensor("u_in", [512, 2048], fp8, kind="ExternalInput").ap()
    # wpack: cols 0:256 me2-dr (i*128+r), cols 256:768 vwt-dr (i*256+c)
    wpack_in = nc.dram_tensor("wpack_in", [128, 768], fp8, kind="ExternalInput").ap()
    wmask_in = nc.dram_tensor("wmask_in", [128, NK * NR], bf16, kind="ExternalInput").ap()
    gw_in = nc.dram_tensor("gw_in", [NR, KH], bf16, kind="ExternalInput").ap()
    out_d = nc.dram_tensor("outp", [C, HW], bf16, kind="ExternalOutput").ap()

    with tile.TileContext(nc) as tc:
        with (
            tc.tile_pool(name="sb", bufs=1) as sb,
            tc.tile_pool(name="ps", bufs=1, space="PSUM") as ps,
        ):
            # ---- dummy exp to prepay the ACT table load ----
            dsrc = sb.tile([128, 1], bf16, tag="dsrc", name="dsrc")
            ddst = sb.tile([128, 1], bf16, tag="ddst", name="ddst")
            nc.vector.memset(dsrc[:], 0.0)
            nc.scalar.activation(ddst[:], dsrc[:], AF.Exp)

            # ---- input DMAs round-robin over the 3 rings ----
            wpk = sb.tile([128, 768], fp8, tag="wpk", name="wpk")
            nc.scalar.dma_start(wpk[:], wpack_in[:, :])
            uc = []
            for ci in range(4):
                t = sb.tile([128, 2048], fp8, tag=f"uc{ci}", name=f"uc{ci}")
                uc.append(t)
            nc.sync.dma_start(uc[0][:], u_in[0:128, :])
            nc.scalar.dma_start(uc[1][:], u_in[128:256, :])
            nc.gpsimd.dma_start(uc[2][:], u_in[256:384, :])
            nc.scalar.dma_start(uc[3][:], u_in[384:512, :])
            wmask = sb.tile([128, NK * NR], bf16, tag="wmask", name="wmask")
            nc.sync.dma_start(wmask[:], wmask_in[:, :])
            gw = sb.tile([NR, KH], bf16, tag="gw", name="gw")
            nc.gpsimd.dma_start(gw[:], gw_in[:, :])

            me2v = wpk[:, 0:256].rearrange("p (i r) -> p i r", i=2)
            vwtv = wpk[:, 256:768].rearrange("p (i c) -> p i c", i=2)

            Fcat = sb.tile([128, HW], bf16, tag="Fcat", name="Fcat")
            SAp = sb.tile([128, 4], f32, tag="SAp", name="SAp")
            SAv = sb.tile([128, 1], f32, tag="SAv", name="SAv")
            SAb = sb.tile([128, 1], bf16, tag="SAb", name="SAb")
            rinv = sb.tile([128, NK], f32, tag="rinv", name="rinv")
            vtb = sb.tile([128, NK * C], bf16, tag="vtb", name="vtb")
            wsc = sb.tile([128, NK * NR], bf16, tag="wsc", name="wsc")
            Wsb = sb.tile([128, C], bf16, tag="Wsb", name="Wsb")
            osb0 = sb.tile([128, HW], bf16, tag="osb0", name="osb0")
            osb1 = sb.tile([128, HW], bf16, tag="osb1", name="osb1")

            # ---- phase 1: E2 + exp (all j), Vt (keys = cols 0:2048) ----
            for ci in range(4):
                ucv = uc[ci].rearrange("p (i x) -> p i x", i=2)
                e2 = ps.tile([128, 1024], f32, tag="big", bufs=2, name=f"e2_{ci}")
                for q in range(2):
                    nc.tensor.matmul(
                        e2[:, q * 512 : (q + 1) * 512], me2v,
                        ucv[:, :, q * 512 : (q + 1) * 512],
                        start=True, stop=True, perf_mode=DR,
                    )
                nc.scalar.activation(
                    Fcat[:, ci * 1024 : (ci + 1) * 1024], e2[:, 0:1024], AF.Exp,
                    scale=1.0 / SC, accum_out=SAp[:, ci : ci + 1],
                )
                if ci < 2:
                    for g in range(4):
                        vt = ps.tile([128, 512], f32, tag="vt", bufs=2,
                                     name=f"vt_{ci}_{g}")
                        for t2 in range(2):
                            kt = ci * 8 + g * 2 + t2
                            lx = (kt % 8) * 128
                            nc.tensor.matmul(
                                vt[:, t2 * 256 : (t2 + 1) * 256],
                                ucv[:, :, lx : lx + 128], vwtv,
                                start=True, stop=True, perf_mode=DR,
                            )
                        kt0 = ci * 8 + g * 2
                        nc.vector.tensor_copy(
                            vtb[:, kt0 * 256 : (kt0 + 2) * 256], vt[:]
                        )

            # ---- softmax denominators: SA -> gather -> 1/s ----
            nc.vector.reduce_sum(SAv[:], SAp[:, 0:4], axis=AX.X)
            nc.vector.tensor_copy(SAb[:], SAv[:])
            sg = ps.tile([128, NK], f32, tag="sg", name="sg")
            for kt in range(NK):
                nc.tensor.matmul(
                    sg[:, kt : kt + 1], gw[:, kt * 128 : (kt + 1) * 128], SAb[:],
                    start=True, stop=True,
                )
            nc.vector.reciprocal(rinv[:], sg[:])

            # ---- W = (wmask/s)^T @ Vt ----
            wps = ps.tile([128, C], f32, tag="w", name="wps")
            for kt in range(NK):
                nc.vector.tensor_scalar(
                    wsc[:, kt * NR : (kt + 1) * NR],
                    wmask[:, kt * NR : (kt + 1) * NR],
                    rinv[:, kt : kt + 1], None, op0=ALU.mult,
                )
                nc.tensor.matmul(
                    wps[:], wsc[:, kt * NR : (kt + 1) * NR],
                    vtb[:, kt * C : (kt + 1) * C],
                    start=(kt == 0), stop=(kt == NK - 1),
                )
            nc.scalar.copy(Wsb[:], wps[:])

            # ---- P = W^T @ Fcat -> bf16 -> DRAM ----
            for ct in range(2):
                osb = osb0 if ct == 0 else osb1
                for ci in range(4):
                    pp = ps.tile([128, 1024], f32, tag="big", bufs=2,
                                 name=f"pp_{ct}_{ci}")
                    for q in range(2):
                        sl = slice(ci * 1024 + q * 512, ci * 1024 + (q + 1) * 512)
                        nc.tensor.matmul(
                            pp[:, q * 512 : (q + 1) * 512],
                            Wsb[:, ct * 128 : (ct + 1) * 128], Fcat[:, sl],
                            start=True, stop=True,
                        )
                    sl = slice(ci * 1024, (ci + 1) * 1024)
                    if (ct * 4 + ci) % 2 == 0:
                        nc.scalar.copy(osb[:, sl], pp[:])
                    else:
                        nc.vector.tensor_copy(osb[:, sl], pp[:])
                    qeng = nc.sync if ct == 0 else nc.gpsimd
                    qeng.dma_start(
                        out_d[ct * 128 : (ct + 1) * 128, sl], osb[:, sl]
                    )

    nc.compile()
    return nc


def _get_built():
    global _BUILT
    if _BUILT is None:
        _BUILT = _build()
    return _BUILT


def _prep_global(boundary_map, key_w1, bn_scale, bn_bias, bn_mean, bn_var,
                 key_w2, query_w, value_w):
    """Segment geometry + weight folding (all float64 host math)."""
    b = boundary_map.shape[0]
    h = 64
    H0 = boundary_map.shape[2]
    idx = (np.arange(h) * H0) // h
    bm = boundary_map[:, 0][:, idx][:, :, idx].reshape(b, HW).astype(np.float64)

    inv = bn_scale.astype(np.float64) / np.sqrt(bn_var.astype(np.float64) + 1e-5)
    beta = bn_bias.astype(np.float64) - bn_mean.astype(np.float64) * inv
    kw1f = key_w1[:, 0].astype(np.float64) * inv
    M = key_w2.astype(np.float64).T @ query_w.astype(np.float64)   # [64, 256]

    tall = bm.reshape(-1)
    lo, hi = tall.min(), tall.max()
    with np.errstate(divide="ignore", invalid="ignore"):
        tstar = np.where(np.abs(kw1f) > 1e-30, -beta / kw1f, np.inf)
    bps = np.sort(tstar[(tstar > lo) & (tstar < hi)])
    edges_true = np.concatenate([[lo], bps, [hi]])
    nseg = len(edges_true) - 1
    assert nseg <= S, f"too many relu segments: {nseg}"

    widths = np.diff(edges_true)
    cnt = np.histogram(tall, bins=edges_true)[0]
    score = widths * np.sqrt(cnt + 1)
    alloc = np.maximum(1, np.floor(score / score.sum() * S).astype(int))
    while alloc.sum() < S:
        alloc[np.argmax(score / alloc)] += 1
    while alloc.sum() > S:
        cand = np.where(alloc > 1)[0]
        alloc[cand[np.argmin((score / alloc)[cand])]] -= 1

    ps_lo = np.empty(S); ps_hi = np.empty(S); ps_true = np.empty(S, np.int64)
    si_out = 0
    for si in range(nseg):
        sub = np.linspace(edges_true[si], edges_true[si + 1], alloc[si] + 1)
        for k in range(alloc[si]):
            ps_lo[si_out] = sub[k]; ps_hi[si_out] = sub[k + 1]
            ps_true[si_out] = si
            si_out += 1
    assert si_out == S

    mids = 0.5 * (edges_true[:-1] + edges_true[1:])
    act = (kw1f[None, :] * mids[:, None] + beta[None, :]) > 0     # [nseg, 64]
    ME2 = np.zeros((NR, 64))
    for s in range(S):
        si = ps_true[s]
        for a, ta in enumerate((ps_lo[s], ps_hi[s])):
            ME2[2 * s + a] = ta * (act[si] * kw1f) + act[si] * beta
    me2m = ME2 @ M                                                # [128, 256]

    seg = np.clip(np.searchsorted(ps_hi, bm, side="left"), 0, S - 1)  # [b, HW]
    width = np.maximum(ps_hi[seg] - ps_lo[seg], 1e-12)
    wl = (ps_hi[seg] - bm) / width
    wh = 1.0 - wl
    return bm, me2m, seg, wl, wh


def _host_prep(boundary_map, uncertainty_map, key_w1, bn_scale, bn_bias,
               bn_mean, bn_var, key_w2, query_w, value_w):
    import ml_dtypes

    bf16 = ml_dtypes.bfloat16
    f8 = ml_dtypes.float8_e4m3
    b, c, h, w = uncertainty_map.shape
    _, me2m, seg, wl, wh = _prep_global(
        boundary_map, key_w1, bn_scale, bn_bias, bn_mean, bn_var,
        key_w2, query_w, value_w,
    )
    # wpack: me2-dr | vwt-dr, both pre-scaled by SC, channel-pair interleaved
    wpack = np.zeros((128, 768), np.float32)
    me2s = (me2m * SC).astype(np.float32)                          # [128r, 256c]
    vws = (value_w.T * SC).astype(np.float32)                      # [256c', 256c]
    for i in range(2):
        wpack[:, i * 128 : (i + 1) * 128] = me2s[:, i::2].T        # [p, r]
        wpack[:, 256 + i * 256 : 256 + (i + 1) * 256] = vws[i::2, :]
    wpack = wpack.astype(f8)

    in_maps = []
    for core in range(8):
        bi, kh = core // 2, core % 2
        u = uncertainty_map[bi].reshape(c, h * w).astype(np.float32)
        if kh == 1:
            u = np.concatenate([u[:, KH:], u[:, :KH]], axis=1)
        u8 = u.astype(f8)
        # chunk blocks: ub[ci*128+p, i*1024+x] = u8[2p+i, ci*1024+x]
        ub = np.ascontiguousarray(
            u8.reshape(128, 2, 4, 1024).transpose(2, 0, 1, 3).reshape(512, 2048)
        )
        ksl = slice(kh * KH, (kh + 1) * KH)
        sg_k = seg[bi, ksl]
        wmask = np.zeros((KH, NR), np.float32)
        kk = np.arange(KH)
        wmask[kk, 2 * sg_k] = wl[bi, ksl]
        wmask[kk, 2 * sg_k + 1] = wh[bi, ksl]
        wm_dev = np.ascontiguousarray(
            wmask.reshape(NK, 128, NR).transpose(1, 0, 2).reshape(128, NK * NR)
        ).astype(bf16)
        gw_dev = np.ascontiguousarray(
            wmask.reshape(NK, 128, NR).transpose(2, 0, 1).reshape(NR, KH)
        ).astype(bf16)
        in_maps.append({
            "u_in": ub,
            "wpack_in": wpack,
            "wmask_in": wm_dev,
            "gw_in": gw_dev,
        })
    return in_maps


def kernel(boundary_map, uncertainty_map, key_w1, bn_scale, bn_bias,
           bn_mean, bn_var, key_w2, query_w, value_w, gamma):
    global LAST_RESULTS
    from concourse.bass_utils import run_bass_kernel_spmd

    nc = _get_built()
    in_maps = _host_prep(
        np.asarray(boundary_map), np.asarray(uncertainty_map), np.asarray(key_w1),
        np.asarray(bn_scale), np.asarray(bn_bias), np.asarray(bn_mean),
        np.asarray(bn_var), np.asarray(key_w2), np.asarray(query_w),
        np.asarray(value_w),
    )
    kwargs = {}
    if TRACE:
        kwargs["trace"] = True
        if TRACE_CORES is not None:
            kwargs["trace_cores"] = TRACE_CORES
    res = run_bass_kernel_spmd(nc, in_maps, core_ids=list(range(8)), **kwargs)
    LAST_RESULTS = res

    b, c, h, w = uncertainty_map.shape
    g = np.float32(np.asarray(gamma).reshape(-1)[0] / SC)
    out = np.empty((b, c, h * w), np.float32)
    um = np.asarray(uncertainty_map)
    for bi in range(b):
        P0 = res.results[2 * bi]["outp"].astype(np.float32)
        P1 = res.results[2 * bi + 1]["outp"].astype(np.float32)
        P1 = np.concatenate([P1[:, KH:], P1[:, :KH]], axis=1)
        out[bi] = g * (P0 + P1) + um[bi].reshape(c, h * w)
    return out.reshape(b, c, h, w)
